# revision 17
# baseline (speedup 1.0000x reference)
"""DeepSpeed-style fused self-attention block for TRN2, tensor-parallel over 8 NeuronCores.

Computes, per reference:
  inp_norm = layernorm(x) * g + b
  qkv      = inp_norm @ qkvw + qkvb ; split into per-head q,k,v (16 heads, dh=128)
  scores   = (q @ k.T) / sqrt(dh), causal + additive mask, softmax
  ctx      = probs @ v
  out      = ctx @ ow
Returns (out, k, v, ctx, inp_norm).

Sharding (DeepSpeed tensor parallel): each core owns 2 heads (qkvw column-sharded,
ow row-sharded); layernorm replicated; `out` partials reduced with a chunked
ReduceScatter across the 8 cores, shards gathered on host.
"""

import os
import sys

import numpy as np

for _p in ("/opt/trn_rl_repo", "/root/.axon_site/_ro/trn_rl_repo", "/root/.axon_site"):
    if os.path.isdir(_p) and _p not in sys.path:
        sys.path.append(_p)

import concourse.bass as bass
import concourse.bacc as bacc
import concourse.mybir as mybir
import concourse.tile as tile
from concourse.bass_utils import run_bass_kernel_spmd

F32 = mybir.dt.float32
F32R = mybir.dt.float32r
AF = mybir.ActivationFunctionType
ALU = mybir.AluOpType

NCORES = 8
HEADS = 16
HPC = HEADS // NCORES  # heads per core = 2
DH = 128
H = 2048
MINUS_INF = -10000.0
LN_EPS = 1e-12

# matmul precision: float32r streams 1 row/cycle (4x faster than float32) at N>=256.
MM_F32R = True
MR = F32R if MM_F32R else F32


def _r(ap):
    return ap


class KernelBuild:
    def __init__(self, nc, ios):
        self.nc = nc
        self.ios = ios


def build_nc(S=2048, B=2, trivial_mask=True, trivial_affine=True):
    """Emit the full Tile kernel for one core (SPMD: all cores run this; per-core
    data arrives via the per-core input maps)."""
    T = B * S  # total tokens
    NT = T // 128  # 128-token tiles
    NCH = T // 256  # 256-token chunks (phase 1)
    NSUP = S // 512  # 512-row q supers per batch
    QKC = HPC * DH  # per-core q (or k, or v) column count = 256

    nc = bacc.Bacc("TRN2", target_bir_lowering=False, debug=False,
                   num_devices=NCORES)

    # ---------------- DRAM I/O ----------------
    x = nc.dram_tensor("x", [T, H], F32, kind="ExternalInput").ap()
    wqkv = nc.dram_tensor("wqkv", [H, 3 * QKC], MR, kind="ExternalInput").ap()
    bqkv = nc.dram_tensor("bqkv", [3 * QKC], F32, kind="ExternalInput").ap()
    oww = nc.dram_tensor("oww", [QKC, H], MR, kind="ExternalInput").ap()
    bkv_r = nc.dram_tensor("bkv_r", [1, 2 * QKC], MR, kind="ExternalInput").ap()
    ones_r = nc.dram_tensor("ones_r", [1, 128], MR, kind="ExternalInput").ap()
    cmask = nc.dram_tensor("cmask", [128, 4, 512], F32, kind="ExternalInput").ap()
    ident = nc.dram_tensor("ident", [128, 128], F32, kind="ExternalInput").ap()
    amask = nc.dram_tensor("amask", [B, S], F32, kind="ExternalInput").ap()
    gamma = nc.dram_tensor("gamma", [H], F32, kind="ExternalInput").ap()
    beta = nc.dram_tensor("beta", [H], F32, kind="ExternalInput").ap()

    inp_norm_o = nc.dram_tensor("inp_norm_o", [T, H], F32, kind="ExternalOutput").ap()
    k_o = nc.dram_tensor("k_o", [B, HPC, S, DH], F32, kind="ExternalOutput").ap()
    v_o = nc.dram_tensor("v_o", [B, HPC, S, DH], F32, kind="ExternalOutput").ap()
    ctx_o = nc.dram_tensor("ctx_o", [T, QKC], F32, kind="ExternalOutput").ap()
    # summed-out shards: T/NCORES rows per core, delivered in T//512 chunks of
    # (512/NCORES) rows each
    NRS = T // 512
    shard_o = nc.dram_tensor("shard_o", [NRS, 512 // NCORES, H], F32,
                             kind="ExternalOutput").ap()

    with tile.TileContext(nc) as tc:
        _body(tc, locals())
    nc.compile()
    return nc


def _body(tc, v):
    nc = tc.nc
    (x, wqkv, bqkv, oww, cmask, ident, amask, gamma, beta, inp_norm_o, k_o, v_o,
     ctx_o, shard_o, bkv_r, ones_r) = (v[k] for k in
                        ("x", "wqkv", "bqkv", "oww", "cmask", "ident", "amask",
                         "gamma", "beta", "inp_norm_o", "k_o", "v_o", "ctx_o",
                         "shard_o", "bkv_r", "ones_r"))
    S, B, T, NT, NCH, NSUP, QKC, NRS = (v[k] for k in
                                        ("S", "B", "T", "NT", "NCH", "NSUP",
                                         "QKC", "NRS"))
    trivial_mask, trivial_affine = v["trivial_mask"], v["trivial_affine"]

    copy_tick = [0]

    def copy(out, in_, scale=None, bias=None):
        """Alternate PSUM->SBUF (and similar) copies between ScalarE and VectorE."""
        copy_tick[0] += 1
        if copy_tick[0] % 2 == 0:
            fn = AF.Copy if (scale is None and bias is None) else AF.Identity
            nc.scalar.activation(out, in_, fn,
                                 scale=1.0 if scale is None else scale,
                                 bias=0.0 if bias is None else bias)
        else:
            if scale is None and bias is None:
                nc.vector.tensor_copy(out, in_)
            else:
                nc.vector.tensor_scalar(
                    out=out, in0=in_,
                    scalar1=1.0 if scale is None else scale,
                    scalar2=0.0 if bias is None else bias,
                    op0=ALU.mult, op1=ALU.add)

    import contextlib
    stack = contextlib.ExitStack()
    with stack:
        persist = stack.enter_context(tc.tile_pool(name="persist", bufs=1))
        dram = stack.enter_context(tc.tile_pool(name="dram", bufs=1, space="DRAM"))

        # ---- persistent SBUF ----
        ident_sb = persist.tile([128, 128], F32)
        nc.sync.dma_start(out=ident_sb, in_=ident)
        eps_sb = persist.tile([128, 1], F32)
        nc.vector.memset(eps_sb, LN_EPS)
        cmask_sb = persist.tile([128, 4, 512], F32)
        nc.sync.dma_start(out=cmask_sb, in_=cmask)
        # v in natural layout [t, hpc*dh], kept resident for the whole kernel
        v_sb = persist.tile([128, NT, QKC], MR)
        if not trivial_mask:
            # broadcast additive mask along partitions: mb_sb[p, b, s] = amask[b, s]
            mb_sb = persist.tile([128, B, S], F32)
            src = bass.AP(tensor=amask.tensor, offset=amask.offset,
                          ap=[[0, 128]] + list(amask.ap))
            nc.sync.dma_start(out=mb_sb, in_=src)
        if not trivial_affine:
            gb_sb = persist.tile([128, H], F32)
            bb_sb = persist.tile([128, H], F32)
            nc.sync.dma_start(out=gb_sb, in_=bass.AP(
                tensor=gamma.tensor, offset=gamma.offset,
                ap=[[0, 128]] + list(gamma.ap)))
            nc.sync.dma_start(out=bb_sb, in_=bass.AP(
                tensor=beta.tensor, offset=beta.offset,
                ap=[[0, 128]] + list(beta.ap)))

        # DRAM scratch
        qkT_dram = dram.tile([2 * QKC, T], MR)  # rows: q cols then k cols
        outpart = dram.tile([T, H], F32)

        # =========== PHASE 1: LN + QKV ===========
        with contextlib.ExitStack() as p1:
            wq_sb = p1.enter_context(tc.tile_pool(name="wq", bufs=1)).tile(
                [128, 16, 3 * QKC], MR)
            nc.sync.dma_start(out=wq_sb,
                              in_=wqkv.rearrange("(ho hi) c -> hi ho c", hi=128))
            bias_pool = p1.enter_context(tc.tile_pool(name="bias", bufs=1))
            # per-partition bias for q|k cols: bqk_pp[p, cc] = bqkv[cc*128+p]
            bqk_pp = bias_pool.tile([128, 2 * QKC // 128], F32)
            nc.sync.dma_start(
                out=bqk_pp,
                in_=bass.AP(tensor=bqkv.tensor, offset=bqkv.offset,
                            ap=[[1, 128], [128, 2 * QKC // 128]]))
            # free-axis bias row for k|v cols (used via K=1 matmul broadcast)
            bkv_row = bias_pool.tile([1, 2 * QKC], MR)
            nc.sync.dma_start(out=bkv_row, in_=bkv_r)
            ones_row = bias_pool.tile([1, 128], MR)
            nc.sync.dma_start(out=ones_row, in_=ones_r)

            xp = p1.enter_context(tc.tile_pool(name="xp", bufs=3))
            lnp = p1.enter_context(tc.tile_pool(name="lnp", bufs=8))
            xcrp = p1.enter_context(tc.tile_pool(name="xcr", bufs=3))
            xtp = p1.enter_context(tc.tile_pool(name="xcrT", bufs=2))
            tp_ps = p1.enter_context(tc.tile_pool(name="tp_ps", bufs=3, space="PSUM"))
            qk_ps = p1.enter_context(tc.tile_pool(name="qk_ps", bufs=2, space="PSUM"))
            kv_ps = p1.enter_context(tc.tile_pool(name="kv_ps", bufs=2, space="PSUM"))
            stg = p1.enter_context(tc.tile_pool(name="stg", bufs=4))

            for ch in range(NCH):  # 256-token chunks
                xcrT = xtp.tile([128, 16, 256], MR)  # transposed normalized x
                for tt in range(2):
                    t0 = ch * 256 + tt * 128
                    xt = xp.tile([128, H], F32)
                    nc.sync.dma_start(out=xt, in_=x[t0:t0 + 128, :])
                    # LN stats over H=2048 via 4x bn_stats + bn_aggr
                    stats = lnp.tile([128, 4, 6], F32)
                    for g4 in range(4):
                        nc.vector.bn_stats(out=stats[:, g4, :],
                                           in_=xt[:, g4 * 512:(g4 + 1) * 512])
                    mv = lnp.tile([128, 2], F32)
                    nc.vector.bn_aggr(out=mv, in_=stats)
                    rstd = lnp.tile([128, 1], F32)
                    nc.scalar.activation(rstd, mv[:, 1:2], AF.Sqrt, bias=eps_sb)
                    nc.vector.reciprocal(rstd, rstd)
                    xcr = xcrp.tile([128, H], F32)
                    # xcr = (x - mean) * rstd
                    nc.vector.tensor_scalar(out=xcr, in0=xt, scalar1=mv[:, 0:1],
                                            scalar2=rstd, op0=ALU.subtract,
                                            op1=ALU.mult)
                    if trivial_affine:
                        nc.sync.dma_start(out=inp_norm_o[t0:t0 + 128, :], in_=xcr)
                    else:
                        xno = xcrp.tile([128, H], F32, tag="xno")
                        nc.vector.tensor_mul(xno, xcr, gb_sb)
                        nc.vector.tensor_add(xno, xno, bb_sb)
                        nc.sync.dma_start(out=inp_norm_o[t0:t0 + 128, :], in_=xno)
                    # transpose 16 h-blocks of xcr -> xcrT[:, :, tt*128:...]
                    for q4 in range(4):
                        ps = tp_ps.tile([128, 512], F32)
                        for hh4 in range(4):
                            hh = q4 * 4 + hh4
                            nc.tensor.transpose(
                                ps[:, hh4 * 128:(hh4 + 1) * 128],
                                xcr[:, hh * 128:(hh + 1) * 128], ident_sb)
                        copy(xcrT[:, q4 * 4:(q4 + 1) * 4, tt * 128:(tt + 1) * 128],
                             ps.rearrange("p (a b) -> p a b", a=4))

                # q,k transposed: psum [128 cols, 256 t]
                for cc in range(2 * QKC // 128):
                    ps = qk_ps.tile([128, 256], F32)
                    for hh in range(16):
                        nc.tensor.matmul(
                            ps, _r(wq_sb[:, hh, cc * 128:(cc + 1) * 128]),
                            _r(xcrT[:, hh, :]),
                            start=(hh == 0), stop=(hh == 15))
                    sg = stg.tile([128, 256], MR, tag="qkstg")
                    nc.scalar.activation(sg, ps, AF.Identity, bias=bqk_pp[:, cc:cc + 1])
                    nc.sync.dma_start(
                        out=qkT_dram[cc * 128:(cc + 1) * 128,
                                     ch * 256:(ch + 1) * 256], in_=sg)
                # k,v natural: psum [128 t, 2*QKC]
                for tt in range(2):
                    t0 = ch * 256 + tt * 128
                    b_i, s0 = t0 // S, t0 % S
                    ps = kv_ps.tile([128, 2 * QKC], F32)
                    for hh in range(16):
                        nc.tensor.matmul(
                            ps, _r(xcrT[:, hh, tt * 128:(tt + 1) * 128]),
                            _r(wq_sb[:, hh, QKC:3 * QKC]),
                            start=(hh == 0), stop=False)
                    nc.tensor.matmul(ps, _r(ones_row), _r(bkv_row),
                                     start=False, stop=True)
                    kst = stg.tile([128, QKC], F32, tag="kstg")
                    copy(kst, ps[:, 0:QKC])
                    nc.sync.dma_start(
                        out=k_o[b_i, :, s0:s0 + 128, :].rearrange("h s d -> s h d"),
                        in_=kst.rearrange("s (h d) -> s h d", h=HPC))
                    copy(v_sb[:, t0 // 128, :], ps[:, QKC:2 * QKC])
                    vst = stg.tile([128, QKC], F32, tag="vstg")
                    copy(vst, ps[:, QKC:2 * QKC])
                    nc.sync.dma_start(
                        out=v_o[b_i, :, s0:s0 + 128, :].rearrange("h s d -> s h d"),
                        in_=vst.rearrange("s (h d) -> s h d", h=HPC))

        # =========== PHASE 2: attention + output projection ===========
        with contextlib.ExitStack() as p2:
            qkp = p2.enter_context(tc.tile_pool(name="qk", bufs=2))
            pp = p2.enter_context(tc.tile_pool(name="pp", bufs=4))
            sml = p2.enter_context(tc.tile_pool(name="sml", bufs=10))
            ptp = p2.enter_context(tc.tile_pool(name="ptp", bufs=4))
            ctxTp = p2.enter_context(tc.tile_pool(name="ctxT", bufs=1))
            sc_ps = p2.enter_context(tc.tile_pool(name="sc_ps", bufs=2, space="PSUM"))
            op_ps = p2.enter_context(tc.tile_pool(name="op_ps", bufs=2, space="PSUM"))
            cx_ps = p2.enter_context(tc.tile_pool(name="cx_ps", bufs=1, space="PSUM"))
            pt_ps = p2.enter_context(tc.tile_pool(name="pt_ps", bufs=2, space="PSUM"))
            ct_ps = p2.enter_context(tc.tile_pool(name="ct_ps", bufs=1, space="PSUM"))
            ocp = p2.enter_context(tc.tile_pool(name="ocp", bufs=4))
            ow_sb = p2.enter_context(tc.tile_pool(name="ow", bufs=1)).tile(
                [128, HPC, H], MR)
            nc.sync.dma_start(out=ow_sb,
                              in_=oww.rearrange("(h d) o -> d h o", d=128))

            for b_i in range(B):
                ctxT_sb = ctxTp.tile([128, HPC, S], MR, tag="ctxT_r")
                if MM_F32R:
                    ctxT_x = ctxTp.tile([128, HPC, S], F32, tag="ctxT_x")
                else:
                    ctxT_x = ctxT_sb
                for hl in range(HPC):
                    qT_sb = qkp.tile([128, S], MR, tag="qT")
                    kT_sb = qkp.tile([128, S], MR, tag="kT")
                    nc.sync.dma_start(
                        out=qT_sb,
                        in_=qkT_dram[hl * 128:(hl + 1) * 128, b_i * S:(b_i + 1) * S])
                    nc.sync.dma_start(
                        out=kT_sb,
                        in_=qkT_dram[QKC + hl * 128:QKC + (hl + 1) * 128,
                                     b_i * S:(b_i + 1) * S])
                    for sp in range(NSUP):
                        W = 512 * (sp + 1)
                        p_tiles, diags = [], []
                        for qi in range(4):
                            q0 = sp * 512 + qi * 128
                            p_i = pp.tile([128, S], MR, tag="p")
                            sum_i = sml.tile([128, 1], F32, tag="sum")
                            for kc in range(sp + 1):
                                ps = sc_ps.tile([128, 512], F32)
                                nc.tensor.matmul(
                                    ps, _r(qT_sb[:, q0:q0 + 128]),
                                    _r(kT_sb[:, kc * 512:(kc + 1) * 512]),
                                    start=True, stop=True)
                                if kc == sp:
                                    nc.vector.tensor_add(ps, ps,
                                                         cmask_sb[:, qi, :])
                                if not trivial_mask:
                                    nc.vector.tensor_add(
                                        ps, ps,
                                        mb_sb[:, b_i, kc * 512:(kc + 1) * 512])
                                if kc == 0:
                                    nc.scalar.activation(
                                        p_i[:, 0:512], ps, AF.Exp,
                                        accum_out=sum_i)
                                else:
                                    s_part = sml.tile([128, 1], F32, tag="spart")
                                    nc.scalar.activation(
                                        p_i[:, kc * 512:(kc + 1) * 512], ps,
                                        AF.Exp, accum_out=s_part)
                                    nc.vector.tensor_add(sum_i, sum_i, s_part)
                            recip = sml.tile([128, 1], F32, tag="recip")
                            nc.vector.reciprocal(recip, sum_i)
                            diag = sml.tile([128, 128], MR, tag="diag")
                            nc.vector.tensor_scalar_mul(diag, ident_sb, recip)
                            p_tiles.append(p_i)
                            diags.append(diag)
                        # transpose+normalize p blocks (pT[j] = [128 kj, 512 qi]),
                        # interleaved with the ctxT accumulation over kj chunks
                        ctps = ct_ps.tile([128, 512], F32)
                        nj = 4 * (sp + 1)
                        for j in range(nj):
                            ptps = pt_ps.tile([128, 512], F32)
                            for qi in range(4):
                                nc.tensor.matmul(
                                    ptps[:, qi * 128:(qi + 1) * 128],
                                    _r(p_tiles[qi][:, j * 128:(j + 1) * 128]),
                                    _r(diags[qi]), start=True, stop=True)
                            pT = ptp.tile([128, 512], MR, tag="pT")
                            copy(pT, ptps)
                            nc.tensor.matmul(
                                ctps,
                                _r(v_sb[:, b_i * (S // 128) + j,
                                        hl * 128:(hl + 1) * 128]),
                                _r(pT),
                                start=(j == 0), stop=(j == nj - 1))
                        copy(ctxT_sb[:, hl, sp * 512:(sp + 1) * 512], ctps)
                        if MM_F32R:
                            copy(ctxT_x[:, hl, sp * 512:(sp + 1) * 512], ctps)

                # ctx output (back-transpose) + output projection for this batch
                for tt in range(S // 128):
                    t0g = b_i * S + tt * 128
                    cps = cx_ps.tile([128, HPC * 128], F32, tag="cx")
                    for hl in range(HPC):
                        nc.tensor.transpose(
                            cps[:, hl * 128:(hl + 1) * 128],
                            ctxT_x[:, hl, tt * 128:(tt + 1) * 128],
                            ident_sb)
                    cxs = ocp.tile([128, QKC], F32, tag="cxs")
                    copy(cxs, cps)
                    nc.sync.dma_start(out=ctx_o[t0g:t0g + 128, :], in_=cxs)
                    for oc in range(4):
                        ops = op_ps.tile([128, 512], F32, tag="ops")
                        for hl in range(HPC):
                            nc.tensor.matmul(
                                ops,
                                _r(ctxT_sb[:, hl, tt * 128:(tt + 1) * 128]),
                                _r(ow_sb[:, hl, oc * 512:(oc + 1) * 512]),
                                start=(hl == 0), stop=(hl == HPC - 1))
                        osb = ocp.tile([128, 512], F32, tag="osb")
                        copy(osb, ops)
                        nc.sync.dma_start(
                            out=outpart[t0g:t0g + 128, oc * 512:(oc + 1) * 512],
                            in_=osb)
                # ReduceScatter the finished 512-row chunks of this batch
                for ci in range(b_i * (S // 512), (b_i + 1) * (S // 512)):
                    sh = dram.tile([512 // NCORES, H], F32, tag="shard")
                    nc.gpsimd.collective_compute(
                        "ReduceScatter", ALU.add,
                        replica_groups=[list(range(NCORES))],
                        ins=[outpart[ci * 512:(ci + 1) * 512, :]],
                        outs=[sh.opt()])
                    nc.sync.dma_start(out=shard_o[ci], in_=sh)


_CACHE = {}


def _get_nc(S, B, trivial_mask, trivial_affine):
    key = (S, B, trivial_mask, trivial_affine)
    if key not in _CACHE:
        _CACHE[key] = build_nc(S, B, trivial_mask, trivial_affine)
    return _CACHE[key]


def host_prep(x, mask, qkvw, qkvb, ow, norm_w, norm_b):
    """Build the 8 per-core input maps."""
    B, S, Hh = x.shape
    T = B * S
    g = norm_w.astype(np.float64)
    bl = norm_b.astype(np.float64)
    scale_q = 1.0 / np.sqrt(DH)
    xf = np.ascontiguousarray(x.reshape(T, Hh).astype(np.float32))
    amask = np.ascontiguousarray(mask.reshape(B, S).astype(np.float32))
    ident = np.eye(128, dtype=np.float32)
    # causal additive patterns: cmask[p, r, j] = 0 if j <= p + 128*r else -inf
    p_i = np.arange(128)[:, None, None]
    r_i = np.arange(4)[None, :, None]
    j_i = np.arange(512)[None, None, :]
    cmaskn = np.where(j_i <= p_i + 128 * r_i, 0.0, MINUS_INF).astype(np.float32)
    cmaskn = np.ascontiguousarray(cmaskn)

    w64 = qkvw.astype(np.float64) * g[:, None]
    b64 = qkvb.astype(np.float64) + bl @ qkvw.astype(np.float64)
    in_maps = []
    for c in range(NCORES):
        lo = c * HPC * DH
        qc = slice(lo, lo + HPC * DH)
        kc = slice(H + lo, H + lo + HPC * DH)
        vc = slice(2 * H + lo, 2 * H + lo + HPC * DH)
        wq = w64[:, qc] * scale_q
        wcat = np.concatenate([wq, w64[:, kc], w64[:, vc]], axis=1)
        bcat = np.concatenate([b64[qc] * scale_q, b64[kc], b64[vc]])
        bcat32 = np.ascontiguousarray(bcat.astype(np.float32))
        in_maps.append({
            "x": xf,
            "wqkv": np.ascontiguousarray(wcat.astype(np.float32)),
            "bqkv": bcat32,
            "bkv_r": np.ascontiguousarray(bcat32[HPC * DH:][None, :]),
            "ones_r": np.ones((1, 128), np.float32),
            "oww": np.ascontiguousarray(ow[qc, :].astype(np.float32)),
            "cmask": cmaskn,
            "ident": ident,
            "amask": amask,
            "gamma": norm_w.astype(np.float32),
            "beta": norm_b.astype(np.float32),
        })
    return in_maps


def assemble(results, B, S):
    """Gather per-core results into full outputs."""
    T = B * S
    out = np.empty((T, H), np.float32)
    NRS = T // 512
    rows = 512 // NCORES
    for ci in range(NRS):
        for c in range(NCORES):
            out[ci * 512 + c * rows:ci * 512 + (c + 1) * rows] = \
                results[c]["shard_o"][ci]
    k = np.empty((B, HEADS, S, DH), np.float32)
    vv = np.empty((B, HEADS, S, DH), np.float32)
    ctx = np.empty((T, H), np.float32)
    for c in range(NCORES):
        k[:, c * HPC:(c + 1) * HPC] = results[c]["k_o"]
        vv[:, c * HPC:(c + 1) * HPC] = results[c]["v_o"]
        ctx[:, c * HPC * DH:(c + 1) * HPC * DH] = results[c]["ctx_o"]
    inp_norm = results[0]["inp_norm_o"]
    return (out.reshape(B, S, H), k, vv, ctx.reshape(B, S, H),
            inp_norm.reshape(B, S, H))


def kernel(x, mask, qkvw, qkvb, ow, norm_w, norm_b, _trace=False):
    x = np.asarray(x)
    B, S, _ = x.shape
    trivial_mask = not np.any(np.asarray(mask))
    trivial_affine = (np.all(np.asarray(norm_w) == 1.0)
                      and not np.any(np.asarray(norm_b)))
    nc = _get_nc(S, B, trivial_mask, trivial_affine)
    in_maps = host_prep(np.asarray(x), np.asarray(mask), np.asarray(qkvw),
                        np.asarray(qkvb), np.asarray(ow), np.asarray(norm_w),
                        np.asarray(norm_b))
    res = run_bass_kernel_spmd(nc, in_maps, list(range(NCORES)), trace=_trace)
    outs = assemble(res.results, B, S)
    if _trace:
        return outs, res
    return outs
